# revision 18
# baseline (speedup 1.0000x reference)
"""Trainium2 Bass kernel for nn_Attention_1503238553757 (LSA attention).

Reference computation (per batch element):
    qkv = x @ w_qkv; q,k,v heads of dim 64
    dots = (q @ k^T) * scale[h]; diagonal masked to -inf
    attn = softmax(dots); out = attn @ v
    y = concat_heads(out) @ w_out + b_out

Sharding: data-parallel over batch (16 batches -> 2 per core x 8 cores).

Per-core schedule (v2 — engine-rebalanced, round-pipelined):
  - scores head pairs emitted adjacently -> PE row-group packing (two K=64
    matmuls run concurrently in row groups 0-63 / 64-127, ~2x score rate)
  - exp is split between the Scalar engine (true exp, per-head scale via
    activation scale AP) and the Vector engine (Schraudolph bit-trick exp:
    bf16 = bitcast(int16(round(A*scale*x + B))), max rel err ~4%, washed
    out by softmax renormalization + diffuse attention averaging)
  - diagonal self-token mask: affine_select on GpSimd (SBUF only)
  - attn@V with (v | ones) stationary -> out^T rows + denominator row in
    PSUM; evacuated to SBUF by ACT/DVE (load-balanced)
  - denominator reciprocal: DMA bounce spreads the [1,N] row to [128,8]
    so reciprocal_approx_fast costs ~8 cycles, then a second bounce
    broadcasts 1/denom to [64,N]; normalize multiply runs on GpSimd
    (all-SBUF), writing osb = yproj lhsT in f16
  - projections (qkv, v, x-transposes, y-proj) are deadline-scheduled
    filler units riding the scores PSUM ring between attention rounds
  - emission is round-based: scores(r) | exp(r) | selects(r) | filler |
    attnV(r-1), so every engine queue follows round order
"""

import os
import sys

for _p in ("/opt/trn_rl_repo", "/root/.axon_site/_ro/trn_rl_repo"):
    if os.path.isdir(_p) and _p not in sys.path:
        sys.path.insert(0, _p)

import numpy as np

import concourse.bass as bass
import concourse.bacc as bacc
import concourse.tile as tile
import concourse.mybir as mybir
from concourse.bass_utils import run_bass_kernel_spmd

# Problem constants (hardcoded per harness contract)
B, N, D = 16, 1024, 512
HEADS, DH = 8, 64
N_CORES = 8
BPC = B // N_CORES  # batches per core = 2

dt = mybir.dt
F32 = dt.float32
BF16 = dt.bfloat16
F16 = dt.float16
I16 = dt.int16
EXP = mybir.ActivationFunctionType.Exp
MUL = mybir.AluOpType.mult
ADD = mybir.AluOpType.add

NT = N // 128   # token tiles = 8
VW = DH + 1     # per-head v width (v | ones)
KD = D // 128   # d/inner k-tiles = 4

# Schraudolph bf16-exp constants (DVE f32->int16 is round-to-nearest,
# verified on HW): exp(x) ~= bitcast_bf16(int16(A16*x + B16))
A16 = 128.0 / float(np.log(2.0))     # 184.6650
B16 = 127.0 * 128.0 - 7.4115         # 16248.59


class EngBal:
    """Static load balancer between the Scalar (act) and Vector (dve)
    engines for PSUM-consuming ops."""

    def __init__(self, nc):
        self.nc = nc
        self.t = {"act": 0.0, "dve": 0.0}

    def pick(self, cost_act, cost_dve):
        if self.t["act"] + cost_act <= self.t["dve"] + cost_dve:
            self.t["act"] += cost_act
            return "act"
        self.t["dve"] += cost_dve
        return "dve"

    def add(self, eng, cost):
        self.t[eng] += cost


def build_program():
    nc = bacc.Bacc("TRN2", target_bir_lowering=False, debug=False,
                   num_devices=N_CORES)

    x = nc.dram_tensor("x", [BPC, N, D], F32, kind="ExternalInput").ap()
    w_qkv = nc.dram_tensor("w_qkv", [D, 3 * D], F32, kind="ExternalInput").ap()
    w_out = nc.dram_tensor("w_out", [D, D], F32, kind="ExternalInput").ap()
    b_out = nc.dram_tensor("b_out", [D], F32, kind="ExternalInput").ap()
    scale = nc.dram_tensor("scale", [HEADS], F32, kind="ExternalInput").ap()
    y = nc.dram_tensor("y", [BPC, N, D], F32, kind="ExternalOutput").ap()

    ident_dram = nc.inline_tensor(np.eye(128, dtype=np.float16), name="ident")

    bal = EngBal(nc)

    import contextlib
    with tile.TileContext(nc) as tc, contextlib.ExitStack() as ctx:
        consts = ctx.enter_context(tc.tile_pool(name="consts", bufs=1))
        p_x = ctx.enter_context(tc.tile_pool(name="p_x", bufs=1))
        p_big = ctx.enter_context(tc.tile_pool(name="p_big", bufs=2))
        p_exp = ctx.enter_context(tc.tile_pool(name="p_exp", bufs=6))
        p_on = ctx.enter_context(tc.tile_pool(name="p_on", bufs=4))
        p_rb = ctx.enter_context(tc.tile_pool(name="p_rb", bufs=4))
        p_sm = ctx.enter_context(tc.tile_pool(name="p_sm", bufs=4))
        p_y = ctx.enter_context(tc.tile_pool(name="p_y", bufs=3))
        psS = ctx.enter_context(tc.tile_pool(name="psS", bufs=2, space="PSUM"))
        psO = ctx.enter_context(tc.tile_pool(name="psO", bufs=2, space="PSUM"))
        p_dram = ctx.enter_context(tc.tile_pool(name="p_dram", bufs=4,
                                                space="DRAM"))

        # ---------------- constants ----------------
        ident_sb = consts.tile([128, 128], F16)
        nc.sync.dma_start(out=ident_sb, in_=ident_dram.ap())
        wqkv_sb = consts.tile([128, KD, 3 * D], F16)
        wout_sb = consts.tile([128, KD, D], F16)
        bout_bc = consts.tile([128, D], F32)
        scale_sb = consts.tile([128, HEADS], F32)
        scale_schr = consts.tile([128, HEADS], F32)

        def emit_const_dmas_early():
            # q/k columns of w_qkv first (prologue critical path)
            nc.gpsimd.dma_start(
                out=wqkv_sb[:, :, 0:2 * D],
                in_=w_qkv.rearrange("(k p) c -> p k c", p=128)[:, :, 0:2 * D],
            )
            nc.gpsimd.dma_start(
                out=wqkv_sb[:, :, 2 * D:3 * D],
                in_=w_qkv.rearrange("(k p) c -> p k c", p=128)[:, :, 2 * D:3 * D],
            )
            nc.sync.dma_start(
                out=bout_bc,
                in_=bass.AP(tensor=b_out.tensor, offset=0,
                            ap=[[0, 128], [1, D]]),
            )
            nc.sync.dma_start(
                out=scale_sb,
                in_=bass.AP(tensor=scale.tensor, offset=0,
                            ap=[[0, 128], [1, HEADS]]),
            )
            nc.vector.tensor_scalar_mul(scale_schr, scale_sb, float(A16))

        def emit_const_dmas_late():
            nc.gpsimd.dma_start(
                out=wout_sb,
                in_=w_out.rearrange("(k p) c -> p k c", p=128),
            )

        # ---------------- per-batch state ----------------
        xT = [None] * BPC
        qkT = [None] * BPC
        vsb = [None] * BPC
        osb = [None] * BPC

        def alloc_batch(b):
            xT[b] = p_big.tile([128, KD, N], F16, tag="xT", name=f"xT{b}")
            qkT[b] = p_big.tile([128, 8, N], F16, tag="qk", name=f"qkT{b}")
            vsb[b] = p_big.tile([128, NT, HEADS * VW + 64], BF16, tag="v",
                                name=f"v{b}")
            osb[b] = p_big.tile([128, KD, N], F16, tag="o", name=f"o{b}")

        def emit_load_x(b):
            x_sb = p_x.tile([128, NT, D], F16, tag="x", name=f"x_sb{b}")
            src = x[b].rearrange("(r p) d -> p r d", p=128)
            for c in range(4):
                nc.gpsimd.dma_start(out=x_sb[:, 2 * c:2 * c + 2, :],
                                    in_=src[:, 2 * c:2 * c + 2, :])
            return x_sb

        def emit_ones(b):
            nc.gpsimd.memset(
                vsb[b][:, :, 0:HEADS * VW].rearrange(
                    "p r (h e) -> p r h e", h=HEADS)[:, :, :, DH:DH + 1],
                1.0,
            )
            nc.gpsimd.memset(vsb[b][:, :, HEADS * VW:], 1.0)

        # ---------------- filler units (ride the psS ring) ----------------
        def evac(dst_ap, src_ap, cost_scale=1.0):
            e = bal.pick(0.67 * cost_scale, 0.73 * cost_scale)
            if e == "act":
                nc.scalar.copy(dst_ap, src_ap)
            else:
                nc.vector.tensor_copy(dst_ap, src_ap)

        # Each unit emits its PE matmuls now and returns a closure for its
        # engine-side work (evac / bias / DMA), run later in the round so
        # the engine queues stay exp-first.
        def u_tr(b, x_sb, kd, half):
            ps_t = psS.tile([128, 512], F16, tag="s",
                            name=f"ps_t{b}_{kd}_{half}")
            for rr in range(4):
                r = 4 * half + rr
                nc.tensor.transpose(
                    ps_t[:, 128 * rr:128 * rr + 128],
                    x_sb[:, r, 128 * kd:128 * kd + 128],
                    ident_sb,
                )
            return lambda: evac(xT[b][:, kd, 512 * half:512 * half + 512], ps_t)

        def u_qk(b, ct, nh):
            ps_qk = psS.tile([128, 512], F32, tag="s",
                             name=f"ps_qk{b}_{ct}_{nh}")
            for kt in range(KD):
                nc.tensor.matmul(
                    ps_qk,
                    wqkv_sb[:, kt, 128 * ct:128 * ct + 128],
                    xT[b][:, kt, 512 * nh:512 * nh + 512],
                    start=(kt == 0), stop=(kt == KD - 1),
                )
            return lambda: evac(qkT[b][:, ct, 512 * nh:512 * nh + 512], ps_qk)

        def u_v(b, rp):
            ps_v = psS.tile([128, N], F32, tag="s", name=f"ps_v{b}_{rp}")
            for rr in range(2):
                for kt in range(KD):
                    nc.tensor.matmul(
                        ps_v[:, 512 * rr:512 * rr + 512],
                        xT[b][:, kt, 128 * (2 * rp + rr):128 * (2 * rp + rr) + 128],
                        wqkv_sb[:, kt, 2 * D:3 * D],
                        start=(kt == 0), stop=(kt == KD - 1),
                    )
            return lambda: evac(
                vsb[b][:, 2 * rp:2 * rp + 2, 0:HEADS * VW].rearrange(
                    "p r (h e) -> p r h e", h=HEADS)[:, :, :, 0:DH],
                ps_v.rearrange("p (r h e) -> p r h e", r=2, h=HEADS),
                cost_scale=2.0,
            )

        def u_yp(b, rp):
            ps_y = psS.tile([128, N], F32, tag="s", name=f"ps_y{b}_{rp}")
            for rr in range(2):
                for kt in range(KD):
                    nc.tensor.matmul(
                        ps_y[:, 512 * rr:512 * rr + 512],
                        osb[b][:, kt, 128 * (2 * rp + rr):128 * (2 * rp + rr) + 128],
                        wout_sb[:, kt, :],
                        start=(kt == 0), stop=(kt == KD - 1),
                    )

            def post():
                for rr in range(2):
                    r = 2 * rp + rr
                    y_sb = p_y.tile([128, D], F32, tag="y")
                    nc.vector.tensor_tensor(
                        y_sb, ps_y[:, 512 * rr:512 * rr + 512], bout_bc, op=ADD)
                    bal.add("dve", 0.73)
                    nc.sync.dma_start(out=y[b, 128 * r:128 * r + 128, :],
                                      in_=y_sb)
            return post

        # ---------------- attention rounds ----------------
        def emit_scores(b, g, jt):
            heads = (2 * g, 2 * g + 1)
            tiles = {}
            for h in heads:
                tiles[h] = psS.tile([128, N], F32, tag="s",
                                    name=f"ps_s{b}_{h}_{jt}")
            for ih in range(2):
                for h in heads:  # adjacent -> row-group packed
                    q_off = (h % 2) * 64
                    nc.tensor.matmul(
                        tiles[h][:, 512 * ih:512 * ih + 512],
                        qkT[b][q_off:q_off + 64, 4 + g, 128 * jt:128 * jt + 128],
                        qkT[b][q_off:q_off + 64, g, 512 * ih:512 * ih + 512],
                        start=True, stop=True,
                    )
            return tiles

        def emit_exp(b, g, jt, s_tiles):
            expts = {}
            for h in sorted(s_tiles):
                expT = p_exp.tile([128, N], BF16, tag="exp",
                                  name=f"expT{b}_{h}_{jt}")
                e = bal.pick(1.10, 1.32)
                if e == "act":
                    nc.scalar.activation(expT, s_tiles[h], EXP,
                                         scale=scale_sb[:, h:h + 1])
                else:
                    nc.vector.tensor_scalar(
                        out=expT.bitcast(I16), in0=s_tiles[h],
                        scalar1=scale_schr[:, h:h + 1], scalar2=float(B16),
                        op0=MUL, op1=ADD,
                    )
                nc.gpsimd.affine_select(
                    out=expT[:, 128 * jt:128 * jt + 128],
                    in_=expT[:, 128 * jt:128 * jt + 128],
                    compare_op=mybir.AluOpType.not_equal,
                    fill=0.0, base=0, channel_multiplier=1,
                    pattern=[[-1, 128]],
                )
                expts[h] = expT
            return expts

        ps_os = {}

        def emit_attnv(b, g, jt, expts):
            heads = (2 * g, 2 * g + 1)
            if jt == 0:
                for h in heads:
                    ps_os[h] = psO.tile([DH + 1, N], F32, tag="o",
                                        name=f"ps_o{b}_{h}")
            for h in heads:
                for ih in range(2):
                    nc.tensor.matmul(
                        ps_os[h][:, 512 * ih:512 * ih + 512],
                        vsb[b][:, jt, VW * h:VW * h + DH + 1],
                        expts[h][:, 512 * ih:512 * ih + 512],
                        start=(jt == 0), stop=(jt == NT - 1),
                    )

        def fp_head(b, g, h, ps_o, tail=False):
            """Evacuate one head's psO, bounce the denominator through DRAM
            for a partition-spread reciprocal, broadcast 1/denom."""
            o_un = p_on.tile([DH + 1, N], F32, tag="on", name=f"o_un{b}_{h}")
            evac(o_un, ps_o, cost_scale=1.6)
            scrB = p_dram.tile([1, N], F32, tag="scrB", name=f"scrB{b}_{h}")
            if tail:
                # latency-optimized: single bounce, reciprocal on the [1,N]
                r_row = p_sm.tile([1, N], F32, tag="rrow", name=f"rrow{b}_{h}")
                nc.vector.reciprocal_approx_fast(r_row, o_un[DH:DH + 1, :])
                nc.sync.dma_start(out=scrB, in_=r_row)
            else:
                scrA = p_dram.tile([1, N], F32, tag="scrA", name=f"scrA{b}_{h}")
                nc.sync.dma_start(out=scrA, in_=o_un[DH:DH + 1, :])
                s128 = p_sm.tile([128, N // 128], F32, tag="s128",
                                 name=f"s128_{b}_{h}")
                nc.sync.dma_start(
                    out=s128,
                    in_=bass.AP(tensor=scrA.tensor, offset=scrA.offset,
                                ap=[[N // 128, 128], [1, N // 128]]),
                )
                r128 = p_sm.tile([128, N // 128], F32, tag="r128",
                                 name=f"r128_{b}_{h}")
                nc.vector.reciprocal_approx_fast(r128, s128)
                bal.add("dve", 0.1)
                nc.sync.dma_start(
                    out=bass.AP(tensor=scrB.tensor, offset=scrB.offset,
                                ap=[[N // 128, 128], [1, N // 128]]),
                    in_=r128,
                )
            rb = p_rb.tile([64, N], F32, tag="rb", name=f"rb{b}_{h}")
            nc.sync.dma_start(
                out=rb,
                in_=bass.AP(tensor=scrB.tensor, offset=scrB.offset,
                            ap=[[0, 64], [1, N]]),
            )
            return o_un, rb

        def fp_norm(b, g, h, o_un, rb, tail=False):
            q_off = (h % 2) * 64
            if tail:
                nc.vector.tensor_tensor(
                    osb[b][q_off:q_off + 64, g, :], o_un[0:DH, :], rb, op=MUL)
            else:
                nc.gpsimd.tensor_tensor(
                    osb[b][q_off:q_off + 64, g, :], o_un[0:DH, :], rb, op=MUL)

        # ================= emission schedule =================
        import functools
        import heapq
        F = functools.partial

        # ---- prologue ----
        x0 = emit_load_x(0)
        emit_const_dmas_early()
        alloc_batch(0)
        for half in range(2):
            for kd in range(KD):
                u_tr(0, x0, kd, half)()
        emit_ones(0)
        for nh in range(2):
            u_qk(0, 0, nh)()
            u_qk(0, 4, nh)()
        u_v(0, 0)()

        # ---- filler queue: (deadline_round, seq, kind, fn) ----
        # kind "pe": fn emits PE matmuls now, returns engine-side closure
        # kind "eng": engine/DMA-only work, runs in the round's post phase
        q = []
        _seq = [0]

        def push(dl, kind, fn):
            heapq.heappush(q, (dl, _seq[0], kind, fn))
            _seq[0] += 1

        for rp in range(1, 4):                        # v(0) r2..7
            push(2 * rp - 2, "pe", F(u_v, 0, rp))
        for p, (ctq, ctk) in enumerate(((1, 5), (2, 6), (3, 7)), start=1):
            push(8 * p - 7, "pe", F(u_qk, 0, ctq, 0))
            push(8 * p - 6, "pe", F(u_qk, 0, ctq, 1))
            push(8 * p - 5, "pe", F(u_qk, 0, ctk, 0))
            push(8 * p - 4, "pe", F(u_qk, 0, ctk, 1))

        x1 = [None]

        def start_b1_load():
            x1[0] = emit_load_x(1)
            emit_const_dmas_late()
            alloc_batch(1)

        push(1, "eng", start_b1_load)
        for i, (kd, half) in enumerate(
                [(kd, half) for half in range(2) for kd in range(KD)]):
            push(7 + i, "pe",
                 F(lambda kd=kd, half=half: u_tr(1, x1[0], kd, half)))
        push(15, "eng", F(emit_ones, 1))
        for rp in range(4):
            push(20 + 2 * rp, "pe", F(u_v, 1, rp))
        push(27, "pe", F(u_qk, 1, 0, 0))
        push(28, "pe", F(u_qk, 1, 0, 1))
        push(29, "pe", F(u_qk, 1, 4, 0))
        push(30, "pe", F(u_qk, 1, 4, 1))
        for p, (ctq, ctk) in enumerate(((1, 5), (2, 6), (3, 7)), start=1):
            push(32 + 8 * p - 7, "pe", F(u_qk, 1, ctq, 0))
            push(32 + 8 * p - 6, "pe", F(u_qk, 1, ctq, 1))
            push(32 + 8 * p - 5, "pe", F(u_qk, 1, ctk, 0))
            push(32 + 8 * p - 4, "pe", F(u_qk, 1, ctk, 1))
        for rp in range(4):                            # yproj(0)
            push(39 + 4 * rp, "pe", F(u_yp, 0, rp))

        # ---- attention rounds with lag-1 attnV ----
        # PE order per round: filler MMs | scores(r) | attnV(r-1); engine
        # order: exp(r) | evacs/finishes.  Keeping >=12 matmuls between a
        # projection and the score tile that reuses its PSUM slot makes the
        # slot-WAR waits pre-satisfied, so score pairs issue back-to-back
        # and row-group-pack.
        rounds = [(b, g, jt) for b in range(BPC) for g in range(4)
                  for jt in range(NT)]
        # Manual scheduling floors: pin every instruction's model-time to its
        # round so the list scheduler keeps the emission order (in particular
        # the alternating row-group score quad, which the PE packs 2x).
        # Floors only bias the static queue order; runtime is semaphore-driven.
        TR = 0.0024   # ms per round
        T0 = 0.013    # prologue span, ms

        def flo(r, d=0.0):
            return tc.tile_wait_until(T0 + r * TR + d)

        prev = None
        for ridx, (b, g, jt) in enumerate(rounds):
            posts, mids = [], []
            with flo(ridx):
                while q and q[0][0] <= ridx:
                    _, _, kind, fn = heapq.heappop(q)
                    if kind == "pe":
                        posts.append(fn())
                    elif kind == "mid":
                        mids.append(fn)
                    else:
                        posts.append(fn)
            with flo(ridx, 0.0003):
                s_tiles = emit_scores(b, g, jt)
            with flo(ridx, 0.0005):
                expts = emit_exp(b, g, jt, s_tiles)
                for m in mids:
                    m()
            if prev is not None:
                pb, pg, pjt, pexp = prev
                with flo(ridx, 0.0010):
                    emit_attnv(pb, pg, pjt, pexp)
                if pjt == NT - 1:
                    # finish the pair: psO evacs must be emitted before the
                    # next pair's attnV allocates the psO ring (mid phase),
                    # after this round's exps (engine queues stay exp-first)
                    def fin(pb=pb, pg=pg, hs=(2 * pg, 2 * pg + 1),
                            pso=dict(ps_os), base=ridx):
                        for h in hs:
                            o_un, rb = fp_head(pb, pg, h, pso[h])
                            push(base + 2, "post",
                                 F(fp_norm, pb, pg, h, o_un, rb))
                    push(ridx, "mid", fin)
            with flo(ridx, 0.0014):
                for p in posts:
                    if p:
                        p()
            prev = (b, g, jt, expts)
        pb, pg, pjt, pexp = prev
        emit_attnv(pb, pg, pjt, pexp)
        while q:
            _, _, kind, fn = heapq.heappop(q)
            r = fn()
            if kind == "pe" and r:
                r()
        # last pair (proven spread-bounce path; norm on DVE for tail latency)
        fins = []
        for h in (2 * pg, 2 * pg + 1):
            fins.append((h,) + fp_head(pb, pg, h, ps_os[h]))
        for h, o_un, rb in fins:
            fp_norm(pb, pg, h, o_un, rb, tail=True)

        # ---- tail: yproj(1) ----
        for rp in range(4):
            u_yp(1, rp)()

    nc.compile()
    return nc


_NC = None


def _get_program():
    global _NC
    if _NC is None:
        _NC = build_program()
    return _NC


def make_in_maps(x, w_qkv, w_out, b_out, scale):
    x = np.ascontiguousarray(np.asarray(x, dtype=np.float32))
    w_qkv = np.ascontiguousarray(np.asarray(w_qkv, dtype=np.float32))
    w_out = np.ascontiguousarray(np.asarray(w_out, dtype=np.float32))
    b_out = np.ascontiguousarray(np.asarray(b_out, dtype=np.float32))
    scale = np.ascontiguousarray(np.asarray(scale, dtype=np.float32))
    return [
        {
            "x": x[c * BPC:(c + 1) * BPC],
            "w_qkv": w_qkv,
            "w_out": w_out,
            "b_out": b_out,
            "scale": scale,
        }
        for c in range(N_CORES)
    ]


def kernel(x, w_qkv, w_out, b_out, scale):
    nc = _get_program()
    in_maps = make_in_maps(x, w_qkv, w_out, b_out, scale)
    res = run_bass_kernel_spmd(nc, in_maps, core_ids=list(range(N_CORES)))
    return np.concatenate([res.results[c]["y"] for c in range(N_CORES)], axis=0)


if __name__ == "__main__":
    rng = np.random.default_rng(0)
    inputs = {
        "x": rng.standard_normal((B, N, D), dtype=np.float32),
        "w_qkv": rng.standard_normal((D, 3 * D), dtype=np.float32) * 0.03,
        "w_out": rng.standard_normal((D, D), dtype=np.float32) * 0.04,
        "b_out": np.zeros(D, dtype=np.float32),
        "scale": np.full(HEADS, DH ** -0.5, dtype=np.float32),
    }
    out = kernel(**inputs)
    print("kernel output", out.shape, out.dtype)


# revision 21
# speedup vs baseline: 1.2608x; 1.2608x over previous
"""Trainium2 Bass kernel for nn_Attention_1503238553757 (LSA attention).

Reference computation (per batch element):
    qkv = x @ w_qkv; q,k,v heads of dim 64
    dots = (q @ k^T) * scale[h]; diagonal masked to -inf
    attn = softmax(dots); out = attn @ v
    y = concat_heads(out) @ w_out + b_out

Sharding: data-parallel over batch (16 batches -> 2 per core x 8 cores).

Per-core schedule (v2 — engine-rebalanced, round-pipelined):
  - scores head pairs emitted adjacently -> PE row-group packing (two K=64
    matmuls run concurrently in row groups 0-63 / 64-127, ~2x score rate)
  - exp is split between the Scalar engine (true exp, per-head scale via
    activation scale AP) and the Vector engine (Schraudolph bit-trick exp:
    bf16 = bitcast(int16(round(A*scale*x + B))), max rel err ~4%, washed
    out by softmax renormalization + diffuse attention averaging)
  - diagonal self-token mask: affine_select on GpSimd (SBUF only)
  - attn@V with (v | ones) stationary -> out^T rows + denominator row in
    PSUM; evacuated to SBUF by ACT/DVE (load-balanced)
  - denominator reciprocal: DMA bounce spreads the [1,N] row to [128,8]
    so reciprocal_approx_fast costs ~8 cycles, then a second bounce
    broadcasts 1/denom to [64,N]; normalize multiply runs on GpSimd
    (all-SBUF), writing osb = yproj lhsT in f16
  - projections (qkv, v, x-transposes, y-proj) are deadline-scheduled
    filler units riding the scores PSUM ring between attention rounds
  - emission is round-based: scores(r) | exp(r) | selects(r) | filler |
    attnV(r-1), so every engine queue follows round order
"""

import os
import sys

for _p in ("/opt/trn_rl_repo", "/root/.axon_site/_ro/trn_rl_repo"):
    if os.path.isdir(_p) and _p not in sys.path:
        sys.path.insert(0, _p)

import numpy as np

import concourse.bass as bass
import concourse.bacc as bacc
import concourse.tile as tile
import concourse.mybir as mybir
from concourse.bass_utils import run_bass_kernel_spmd

# Problem constants (hardcoded per harness contract)
B, N, D = 16, 1024, 512
HEADS, DH = 8, 64
N_CORES = 8
BPC = B // N_CORES  # batches per core = 2

dt = mybir.dt
F32 = dt.float32
BF16 = dt.bfloat16
F16 = dt.float16
I16 = dt.int16
EXP = mybir.ActivationFunctionType.Exp
MUL = mybir.AluOpType.mult
ADD = mybir.AluOpType.add

NT = N // 128   # token tiles = 8
VW = DH + 1     # per-head v width (v | ones)
KD = D // 128   # d/inner k-tiles = 4

# Schraudolph bf16-exp constants (DVE f32->int16 is round-to-nearest,
# verified on HW): exp(x) ~= bitcast_bf16(int16(A16*x + B16))
A16 = 128.0 / float(np.log(2.0))     # 184.6650
B16 = 127.0 * 128.0 - 7.4115         # 16248.59


class EngBal:
    """Static load balancer between the Scalar (act) and Vector (dve)
    engines for PSUM-consuming ops."""

    def __init__(self, nc):
        self.nc = nc
        self.t = {"act": 0.0, "dve": 0.0}

    def pick(self, cost_act, cost_dve):
        if self.t["act"] + cost_act <= self.t["dve"] + cost_dve:
            self.t["act"] += cost_act
            return "act"
        self.t["dve"] += cost_dve
        return "dve"

    def add(self, eng, cost):
        self.t[eng] += cost


def build_program():
    nc = bacc.Bacc("TRN2", target_bir_lowering=False, debug=False,
                   num_devices=N_CORES)

    x = nc.dram_tensor("x", [BPC, N, D], F32, kind="ExternalInput").ap()
    w_qkv = nc.dram_tensor("w_qkv", [D, 3 * D], F32, kind="ExternalInput").ap()
    w_out = nc.dram_tensor("w_out", [D, D], F32, kind="ExternalInput").ap()
    b_out = nc.dram_tensor("b_out", [D], F32, kind="ExternalInput").ap()
    scale = nc.dram_tensor("scale", [HEADS], F32, kind="ExternalInput").ap()
    y = nc.dram_tensor("y", [BPC, N, D], F32, kind="ExternalOutput").ap()

    ident_dram = nc.inline_tensor(np.eye(128, dtype=np.float16), name="ident")

    bal = EngBal(nc)

    import contextlib
    with tile.TileContext(nc) as tc, contextlib.ExitStack() as ctx:
        consts = ctx.enter_context(tc.tile_pool(name="consts", bufs=1))
        p_x = ctx.enter_context(tc.tile_pool(name="p_x", bufs=1))
        p_big = ctx.enter_context(tc.tile_pool(name="p_big", bufs=2))
        p_exp = ctx.enter_context(tc.tile_pool(name="p_exp", bufs=6))
        p_on = ctx.enter_context(tc.tile_pool(name="p_on", bufs=4))
        p_rb = ctx.enter_context(tc.tile_pool(name="p_rb", bufs=4))
        p_sm = ctx.enter_context(tc.tile_pool(name="p_sm", bufs=4))
        p_y = ctx.enter_context(tc.tile_pool(name="p_y", bufs=3))
        psS = ctx.enter_context(tc.tile_pool(name="psS", bufs=2, space="PSUM"))
        psO = ctx.enter_context(tc.tile_pool(name="psO", bufs=2, space="PSUM"))
        p_dram = ctx.enter_context(tc.tile_pool(name="p_dram", bufs=4,
                                                space="DRAM"))

        # ---------------- constants ----------------
        ident_sb = consts.tile([128, 128], F16)
        nc.sync.dma_start(out=ident_sb, in_=ident_dram.ap())
        wqkv_sb = consts.tile([128, KD, 3 * D], F16)
        wout_sb = consts.tile([128, KD, D], F16)
        bout_bc = consts.tile([128, D], F32)
        scale_sb = consts.tile([128, HEADS], F32)
        scale_schr = consts.tile([128, HEADS], F32)

        def emit_const_dmas_early():
            # q/k columns of w_qkv first (prologue critical path)
            nc.gpsimd.dma_start(
                out=wqkv_sb[:, :, 0:2 * D],
                in_=w_qkv.rearrange("(k p) c -> p k c", p=128)[:, :, 0:2 * D],
            )
            nc.gpsimd.dma_start(
                out=wqkv_sb[:, :, 2 * D:3 * D],
                in_=w_qkv.rearrange("(k p) c -> p k c", p=128)[:, :, 2 * D:3 * D],
            )
            nc.sync.dma_start(
                out=bout_bc,
                in_=bass.AP(tensor=b_out.tensor, offset=0,
                            ap=[[0, 128], [1, D]]),
            )
            nc.sync.dma_start(
                out=scale_sb,
                in_=bass.AP(tensor=scale.tensor, offset=0,
                            ap=[[0, 128], [1, HEADS]]),
            )
            nc.vector.tensor_scalar_mul(scale_schr, scale_sb, float(A16))

        def emit_const_dmas_late():
            nc.gpsimd.dma_start(
                out=wout_sb,
                in_=w_out.rearrange("(k p) c -> p k c", p=128),
            )

        # ---------------- per-batch state ----------------
        xT = [None] * BPC
        qkT = [None] * BPC
        vsb = [None] * BPC
        osb = [None] * BPC

        def alloc_batch(b):
            xT[b] = p_big.tile([128, KD, N], F16, tag="xT", name=f"xT{b}")
            qkT[b] = p_big.tile([128, 8, N], F16, tag="qk", name=f"qkT{b}")
            vsb[b] = p_big.tile([128, NT, HEADS * VW + 64], BF16, tag="v",
                                name=f"v{b}")
            osb[b] = p_big.tile([128, KD, N], F16, tag="o", name=f"o{b}")

        def emit_load_x(b):
            x_sb = p_x.tile([128, NT, D], F16, tag="x", name=f"x_sb{b}")
            src = x[b].rearrange("(r p) d -> p r d", p=128)
            for c in range(4):
                nc.gpsimd.dma_start(out=x_sb[:, 2 * c:2 * c + 2, :],
                                    in_=src[:, 2 * c:2 * c + 2, :])
            return x_sb

        def emit_ones(b):
            nc.gpsimd.memset(
                vsb[b][:, :, 0:HEADS * VW].rearrange(
                    "p r (h e) -> p r h e", h=HEADS)[:, :, :, DH:DH + 1],
                1.0,
            )
            nc.gpsimd.memset(vsb[b][:, :, HEADS * VW:], 1.0)

        # ---------------- filler units (ride the psS ring) ----------------
        def evac(dst_ap, src_ap, cost_scale=1.0):
            e = bal.pick(0.67 * cost_scale, 0.73 * cost_scale)
            if e == "act":
                nc.scalar.copy(dst_ap, src_ap)
            else:
                nc.vector.tensor_copy(dst_ap, src_ap)

        # Each unit emits its PE matmuls now and returns a closure for its
        # engine-side work (evac / bias / DMA), run later in the round so
        # the engine queues stay exp-first.
        def u_tr(b, x_sb, kd, half):
            ps_t = psS.tile([128, 512], F16, tag="s",
                            name=f"ps_t{b}_{kd}_{half}")
            for rr in range(4):
                r = 4 * half + rr
                nc.tensor.transpose(
                    ps_t[:, 128 * rr:128 * rr + 128],
                    x_sb[:, r, 128 * kd:128 * kd + 128],
                    ident_sb,
                )
            evac(xT[b][:, kd, 512 * half:512 * half + 512], ps_t)

        def u_qk(b, ct, nh):
            ps_qk = psS.tile([128, 512], F32, tag="s",
                             name=f"ps_qk{b}_{ct}_{nh}")
            for kt in range(KD):
                nc.tensor.matmul(
                    ps_qk,
                    wqkv_sb[:, kt, 128 * ct:128 * ct + 128],
                    xT[b][:, kt, 512 * nh:512 * nh + 512],
                    start=(kt == 0), stop=(kt == KD - 1),
                )
            evac(qkT[b][:, ct, 512 * nh:512 * nh + 512], ps_qk)

        def u_v(b, r):
            ps_v = psS.tile([128, 512], F32, tag="s", name=f"ps_v{b}_{r}")
            for kt in range(KD):
                nc.tensor.matmul(
                    ps_v,
                    xT[b][:, kt, 128 * r:128 * r + 128],
                    wqkv_sb[:, kt, 2 * D:3 * D],
                    start=(kt == 0), stop=(kt == KD - 1),
                )
            evac(
                vsb[b][:, r, 0:HEADS * VW].rearrange(
                    "p (h e) -> p h e", h=HEADS)[:, :, 0:DH],
                ps_v.rearrange("p (h e) -> p h e", h=HEADS),
            )

        def u_yp(b, r):
            ps_y = psS.tile([128, 512], F32, tag="s", name=f"ps_y{b}_{r}")
            for kt in range(KD):
                nc.tensor.matmul(
                    ps_y,
                    osb[b][:, kt, 128 * r:128 * r + 128],
                    wout_sb[:, kt, :],
                    start=(kt == 0), stop=(kt == KD - 1),
                )
            y_sb = p_y.tile([128, D], F32, tag="y")
            nc.vector.tensor_tensor(y_sb, ps_y, bout_bc, op=ADD)
            bal.add("dve", 0.73)
            nc.sync.dma_start(out=y[b, 128 * r:128 * r + 128, :], in_=y_sb)

        # ---------------- attention rounds ----------------
        def emit_scores(b, g, jt):
            heads = (2 * g, 2 * g + 1)
            tiles = {}
            for h in heads:
                tiles[h] = psS.tile([128, N], F32, tag="s",
                                    name=f"ps_s{b}_{h}_{jt}")
            for ih in range(2):
                for h in heads:  # adjacent -> row-group packed
                    q_off = (h % 2) * 64
                    nc.tensor.matmul(
                        tiles[h][:, 512 * ih:512 * ih + 512],
                        qkT[b][q_off:q_off + 64, 4 + g, 128 * jt:128 * jt + 128],
                        qkT[b][q_off:q_off + 64, g, 512 * ih:512 * ih + 512],
                        start=True, stop=True,
                    )
            return tiles

        def emit_exp(b, g, jt, s_tiles):
            expts = {}
            for h in sorted(s_tiles):
                expT = p_exp.tile([128, N], BF16, tag="exp",
                                  name=f"expT{b}_{h}_{jt}")
                e = bal.pick(1.10, 1.32)
                if e == "act":
                    nc.scalar.activation(expT, s_tiles[h], EXP,
                                         scale=scale_sb[:, h:h + 1])
                else:
                    nc.vector.tensor_scalar(
                        out=expT.bitcast(I16), in0=s_tiles[h],
                        scalar1=scale_schr[:, h:h + 1], scalar2=float(B16),
                        op0=MUL, op1=ADD,
                    )
                nc.gpsimd.affine_select(
                    out=expT[:, 128 * jt:128 * jt + 128],
                    in_=expT[:, 128 * jt:128 * jt + 128],
                    compare_op=mybir.AluOpType.not_equal,
                    fill=0.0, base=0, channel_multiplier=1,
                    pattern=[[-1, 128]],
                )
                expts[h] = expT
            return expts

        ps_os = {}

        def emit_attnv(b, g, jt, expts):
            heads = (2 * g, 2 * g + 1)
            if jt == 0:
                for h in heads:
                    ps_os[h] = psO.tile([DH + 1, N], F32, tag="o",
                                        name=f"ps_o{b}_{h}")
            for h in heads:
                for ih in range(2):
                    nc.tensor.matmul(
                        ps_os[h][:, 512 * ih:512 * ih + 512],
                        vsb[b][:, jt, VW * h:VW * h + DH + 1],
                        expts[h][:, 512 * ih:512 * ih + 512],
                        start=(jt == 0), stop=(jt == NT - 1),
                    )

        def fp_head(b, g, h, ps_o, tail=False):
            """Evacuate one head's psO, bounce the denominator through DRAM
            for a partition-spread reciprocal, broadcast 1/denom."""
            o_un = p_on.tile([DH + 1, N], F32, tag="on", name=f"o_un{b}_{h}")
            evac(o_un, ps_o, cost_scale=1.6)
            scrB = p_dram.tile([1, N], F32, tag="scrB", name=f"scrB{b}_{h}")
            if tail:
                # latency-optimized: single bounce, reciprocal on the [1,N]
                r_row = p_sm.tile([1, N], F32, tag="rrow", name=f"rrow{b}_{h}")
                nc.vector.reciprocal_approx_fast(r_row, o_un[DH:DH + 1, :])
                nc.sync.dma_start(out=scrB, in_=r_row)
            else:
                scrA = p_dram.tile([1, N], F32, tag="scrA", name=f"scrA{b}_{h}")
                nc.sync.dma_start(out=scrA, in_=o_un[DH:DH + 1, :])
                s128 = p_sm.tile([128, N // 128], F32, tag="s128",
                                 name=f"s128_{b}_{h}")
                nc.sync.dma_start(
                    out=s128,
                    in_=bass.AP(tensor=scrA.tensor, offset=scrA.offset,
                                ap=[[N // 128, 128], [1, N // 128]]),
                )
                r128 = p_sm.tile([128, N // 128], F32, tag="r128",
                                 name=f"r128_{b}_{h}")
                nc.vector.reciprocal_approx_fast(r128, s128)
                bal.add("dve", 0.1)
                nc.sync.dma_start(
                    out=bass.AP(tensor=scrB.tensor, offset=scrB.offset,
                                ap=[[N // 128, 128], [1, N // 128]]),
                    in_=r128,
                )
            rb = p_rb.tile([64, N], F32, tag="rb", name=f"rb{b}_{h}")
            nc.sync.dma_start(
                out=rb,
                in_=bass.AP(tensor=scrB.tensor, offset=scrB.offset,
                            ap=[[0, 64], [1, N]]),
            )
            return o_un, rb

        def fp_norm(b, g, h, o_un, rb, tail=False):
            q_off = (h % 2) * 64
            if tail:
                nc.vector.tensor_tensor(
                    osb[b][q_off:q_off + 64, g, :], o_un[0:DH, :], rb, op=MUL)
            else:
                nc.gpsimd.tensor_tensor(
                    osb[b][q_off:q_off + 64, g, :], o_un[0:DH, :], rb, op=MUL)

        # ================= emission schedule =================
        import functools
        import heapq
        F = functools.partial

        # ---- prologue ----
        x0 = emit_load_x(0)
        emit_const_dmas_early()
        alloc_batch(0)
        for half in range(2):
            for kd in range(KD):
                u_tr(0, x0, kd, half)
        emit_ones(0)
        for nh in range(2):
            u_qk(0, 0, nh)
            u_qk(0, 4, nh)
        u_v(0, 0)
        u_v(0, 1)

        # ---- filler queue: (deadline_round, seq, emit_fn) ----
        q = []
        _seq = [0]

        def push(dl, fn):
            heapq.heappush(q, (dl, _seq[0], fn))
            _seq[0] += 1

        for r in range(2, NT):                        # v(0) r2..7
            push(r - 1, F(u_v, 0, r))
        for p, (ctq, ctk) in enumerate(((1, 5), (2, 6), (3, 7)), start=1):
            push(8 * p - 6, F(u_qk, 0, ctq, 0))
            push(8 * p - 5, F(u_qk, 0, ctq, 1))
            push(8 * p - 4, F(u_qk, 0, ctk, 0))
            push(8 * p - 3, F(u_qk, 0, ctk, 1))

        x1 = [None]

        def start_b1_load():
            x1[0] = emit_load_x(1)
            emit_const_dmas_late()
            alloc_batch(1)

        push(1, start_b1_load)
        for i, (kd, half) in enumerate(
                [(kd, half) for half in range(2) for kd in range(KD)]):
            push(6 + i, F(lambda kd=kd, half=half: u_tr(1, x1[0], kd, half)))
        push(14, F(emit_ones, 1))
        push(22, F(u_v, 1, 0))
        push(23, F(u_v, 1, 1))
        push(25, F(u_qk, 1, 0, 0))
        push(26, F(u_qk, 1, 0, 1))
        push(27, F(u_qk, 1, 4, 0))
        push(28, F(u_qk, 1, 4, 1))
        for i, r in enumerate(range(2, NT)):          # v(1) r2..7
            push(29 + i // 2, F(u_v, 1, r))
        for p, (ctq, ctk) in enumerate(((1, 5), (2, 6), (3, 7)), start=1):
            push(32 + 8 * p - 6, F(u_qk, 1, ctq, 0))
            push(32 + 8 * p - 5, F(u_qk, 1, ctq, 1))
            push(32 + 8 * p - 4, F(u_qk, 1, ctk, 0))
            push(32 + 8 * p - 3, F(u_qk, 1, ctk, 1))
        for i in range(NT):                            # yproj(0)
            push(39 + 2 * i, F(u_yp, 0, i))

        def emit_finish_pair(b, g, tail=False):
            fins = []
            for h in (2 * g, 2 * g + 1):
                fins.append((h,) + fp_head(b, g, h, ps_os[h]))
            for h, o_un, rb in fins:
                fp_norm(b, g, h, o_un, rb, tail=tail)

        # ---- attention rounds with lag-1 attnV ----
        rounds = [(b, g, jt) for b in range(BPC) for g in range(4)
                  for jt in range(NT)]
        prev = None
        for ridx, (b, g, jt) in enumerate(rounds):
            s_tiles = emit_scores(b, g, jt)
            expts = emit_exp(b, g, jt, s_tiles)
            while q and q[0][0] <= ridx:
                heapq.heappop(q)[2]()
            if prev is not None:
                pb, pg, pjt, pexp = prev
                emit_attnv(pb, pg, pjt, pexp)
                if pjt == NT - 1:
                    emit_finish_pair(pb, pg)
            prev = (b, g, jt, expts)
        pb, pg, pjt, pexp = prev
        emit_attnv(pb, pg, pjt, pexp)
        while q:
            heapq.heappop(q)[2]()
        emit_finish_pair(pb, pg, tail=True)

        # ---- tail: yproj(1) ----
        for r in range(NT):
            u_yp(1, r)

    nc.compile()
    return nc


_NC = None


def _get_program():
    global _NC
    if _NC is None:
        _NC = build_program()
    return _NC


def make_in_maps(x, w_qkv, w_out, b_out, scale):
    x = np.ascontiguousarray(np.asarray(x, dtype=np.float32))
    w_qkv = np.ascontiguousarray(np.asarray(w_qkv, dtype=np.float32))
    w_out = np.ascontiguousarray(np.asarray(w_out, dtype=np.float32))
    b_out = np.ascontiguousarray(np.asarray(b_out, dtype=np.float32))
    scale = np.ascontiguousarray(np.asarray(scale, dtype=np.float32))
    return [
        {
            "x": x[c * BPC:(c + 1) * BPC],
            "w_qkv": w_qkv,
            "w_out": w_out,
            "b_out": b_out,
            "scale": scale,
        }
        for c in range(N_CORES)
    ]


def kernel(x, w_qkv, w_out, b_out, scale):
    nc = _get_program()
    in_maps = make_in_maps(x, w_qkv, w_out, b_out, scale)
    res = run_bass_kernel_spmd(nc, in_maps, core_ids=list(range(N_CORES)))
    return np.concatenate([res.results[c]["y"] for c in range(N_CORES)], axis=0)


if __name__ == "__main__":
    rng = np.random.default_rng(0)
    inputs = {
        "x": rng.standard_normal((B, N, D), dtype=np.float32),
        "w_qkv": rng.standard_normal((D, 3 * D), dtype=np.float32) * 0.03,
        "w_out": rng.standard_normal((D, D), dtype=np.float32) * 0.04,
        "b_out": np.zeros(D, dtype=np.float32),
        "scale": np.full(HEADS, DH ** -0.5, dtype=np.float32),
    }
    out = kernel(**inputs)
    print("kernel output", out.shape, out.dtype)
